# revision 4
# baseline (speedup 1.0000x reference)
"""NVFP4Linear (fake-quant e2m1 matmul + bias) on 8 Trainium2 NeuronCores.

Strategy
--------
y = fake_quant_e2m1(x) @ fake_quant_e2m1(w).T + bias, scales are all ones.

e2m1 quantization with the reference's tie-down rounding is a 7-step odd
staircase.  We compute V = sign(x) * T(x) with integer T in {0..12} (the
e2m1 magnitude grid is {0,.5,1,1.5,2,3,4,6} = 0.5 * {0,1,2,3,4,6,8,12}),
using three fused custom DVE ops that compare s = x^2 against squared
midpoints (exact in fp32 for bf16 inputs, strict '>' reproduces the
reference's ties-down behaviour):

  Q1: P  = (s>.0625) + (s>.5625) + (s>1.5625)
  Q2: T2 = P + (s>3.0625) + 2*((s>6.25) + (s>12.25))
  Q3: V  = sgn(x) * (T2 + 4*(s>25))

The global 0.25 factor (0.5 per operand) is applied at PSUM eviction;
bias is accumulated exactly into the fp32 PSUM via a K=1 matmul against
4*bias, so the result matches the fp32 reference bit-for-bit.

Sharding: data-parallel over M (1024 rows/core).  Each core quantizes a
1/8 shard of the weight (512 out-features), the quantized shards are
AllGathered on-device, and each core then runs its [1024,4096]x[4096,4096]
matmul.  Inputs are fed pre-transposed (K-major) from the host so no
on-device transposes are needed.
"""

import numpy as np
import ml_dtypes

import concourse.bass as bass
import concourse.mybir as mybir
import concourse.tile as tile
from concourse import bacc
from concourse import bass_utils
import concourse.dve_ops as dve_ops
from concourse.dve_ops import DveOp
from concourse.dve_spec import Spec, Src0, Src1, C0, C1, C2, Zero, One, sq, lower
from concourse.dve_uop import DveOpSpec

BF16 = ml_dtypes.bfloat16

M, K, N, NC = 8192, 4096, 4096, 8
MIDS_SQ = (0.0625, 0.5625, 1.5625, 3.0625, 6.25, 12.25, 25.0)

# ---------------------------------------------------------------- custom ops


def _np_f32(a):
    return np.asarray(a, np.float32)


def _q1_ref(in0, in1, c0, c1, c2):
    s = _np_f32(in0) ** 2
    return (s > c0).astype(np.float32) + (s > c1) + (s > c2)


def _q2_ref(in0, in1, c0, c1, c2):
    s = _np_f32(in0) ** 2
    u = (s > c1).astype(np.float32) + (s > c2)
    return _np_f32(in1) + (s > c0) + u + u


def _q3_ref(in0, in1, c0, c1, c2):
    s = _np_f32(in0) ** 2
    t = _np_f32(in1) + (s > c0) * c1
    neg = (_np_f32(in0) < 0).astype(np.float32)
    return t - 2 * neg * t


_QOPS = {}


def _register_quant_ops():
    if _QOPS:
        return _QOPS
    s = sq(Src0)
    u = (s > C1) + (s > C2)
    t3 = Src1 + (s > C0) * C1
    neg = Src0 < Zero
    n_t = neg * t3
    bodies = [
        ("E2M1_Q1_ANT", Spec(body=(s > C0) + (s > C1) + (s > C2), reference=_q1_ref), False),
        ("E2M1_Q2_ANT", Spec(body=(Src1 + (s > C0)) + (u + u), reference=_q2_ref), True),
        ("E2M1_Q3_ANT", Spec(body=(t3 - n_t) - n_t, reference=_q3_ref), True),
    ]
    for name, spec, rd1 in bodies:
        if name in dve_ops._SUB_OPCODE_FOR_NAME:
            _QOPS[name] = next(o for o in dve_ops.OPS if o.name == name)
            continue
        shas = {}
        for ver in ("v3", "v4"):
            try:
                uops = lower(spec, ver=ver)
                shas[ver] = DveOpSpec(name=name, opcode=0, uops=uops, rd1_en=rd1).sha(ver)
            except Exception:
                pass
        op = DveOp(name, spec, subdim=False, uops_sha=shas)
        dve_ops.OPS.append(op)
        dve_ops._SUB_OPCODE_FOR_NAME[name] = (
            dve_ops._CUSTOM_DVE_ROW_BASE + len(dve_ops.OPS) - 1
        )
        dve_ops.CUSTOM_DVE_SPECS[name] = spec
        _QOPS[name] = op
    return _QOPS


# ---------------------------------------------------------------- program


def _emit_quant(nc, pools, raw_ap, out_ap):
    """raw_ap -> out_ap, both [128, FD] SBUF bf16 APs. 3 custom DVE passes."""
    ops = _QOPS
    p, fd = raw_ap.shape
    t1 = pools["qtmp1"].tile([p, fd], mybir.dt.bfloat16, tag="qtmp1", name=f"qt1_{nc.next_id()}")
    t2 = pools["qtmp2"].tile([p, fd], mybir.dt.bfloat16, tag="qtmp2", name=f"qt2_{nc.next_id()}")
    nc.vector._custom_dve(ops["E2M1_Q1_ANT"], out=t1[:], in0=raw_ap,
                          s0=MIDS_SQ[0], s1=MIDS_SQ[1], imm2=MIDS_SQ[2])
    nc.vector._custom_dve(ops["E2M1_Q2_ANT"], out=t2[:], in0=raw_ap, in1=t1[:],
                          s0=MIDS_SQ[3], s1=MIDS_SQ[4], imm2=MIDS_SQ[5])
    nc.vector._custom_dve(ops["E2M1_Q3_ANT"], out=out_ap, in0=raw_ap, in1=t2[:],
                          s0=MIDS_SQ[6], s1=4.0)


def build_program(m_sh=M // NC, k=K, n=N, nc_cores=NC, n_strip=512):
    """Build the per-core Bass program (identical on all cores)."""
    _register_quant_ops()
    n_sh = n // nc_cores           # weight rows quantized per core
    kt = k // 128                  # K tiles
    mt = m_sh // 128               # M tiles per core
    nst = n // n_strip             # N strips
    wq_kt = n_sh and (n_sh // 128)

    nc = bacc.Bacc("TRN2", target_bir_lowering=False, debug=False,
                   enable_asserts=False, num_devices=nc_cores)

    xT = nc.dram_tensor("xT", [k, m_sh], mybir.dt.bfloat16, kind="ExternalInput")
    wT = nc.dram_tensor("wT", [k, n_sh], mybir.dt.bfloat16, kind="ExternalInput")
    bias4 = nc.dram_tensor("bias4", [1, n], mybir.dt.bfloat16, kind="ExternalInput")
    out = nc.dram_tensor("out", [m_sh, n], mybir.dt.bfloat16, kind="ExternalOutput")

    with tile.TileContext(nc) as tc:
        with (
            tc.tile_pool(name="dram", bufs=1, space="DRAM") as dram,
            tc.tile_pool(name="wraw", bufs=3) as wraw_pool,
            tc.tile_pool(name="qtmp1", bufs=3) as qtmp1_pool,
            tc.tile_pool(name="qtmp2", bufs=3) as qtmp2_pool,
            tc.tile_pool(name="vw", bufs=3) as vw_pool,
            tc.tile_pool(name="xraw", bufs=3) as xraw_pool,
            tc.tile_pool(name="vx", bufs=kt) as vx_pool,
            tc.tile_pool(name="rhs", bufs=2 * kt) as rhs_pool,
            tc.tile_pool(name="psum", bufs=4, space="PSUM") as psum_pool,
            tc.tile_pool(name="osb", bufs=4) as osb_pool,
            tc.tile_pool(name="const", bufs=1) as const_pool,
        ):
            pools = {"qtmp1": qtmp1_pool, "qtmp2": qtmp2_pool}

            # constants
            ones_t = const_pool.tile([1, 128], mybir.dt.bfloat16, name="ones_t")
            nc.any.memset(ones_t[:], 1.0)
            bias_t = const_pool.tile([1, n], mybir.dt.bfloat16, name="bias_t")
            nc.sync.dma_start(bias_t[:], bias4[:, :])

            # ---- quantize local weight shard, transposed layout [k, n_sh]
            w_bounce = dram.tile([k, n_sh], mybir.dt.bfloat16, name="w_bounce")
            w_full = dram.tile([k * nc_cores, n_sh], mybir.dt.bfloat16,
                               name="w_full",
                               addr_space="Shared" if nc_cores > 4 else "Local")
            for s in range(kt):
                wr = wraw_pool.tile([128, n_sh], mybir.dt.bfloat16, tag="wraw",
                                    name=f"wr_{s}")
                nc.sync.dma_start(wr[:], wT[s * 128:(s + 1) * 128, :])
                vw = vw_pool.tile([128, n_sh], mybir.dt.bfloat16, tag="vw",
                                  name=f"vw_{s}")
                _emit_quant(nc, pools, wr[:], vw[:])
                nc.sync.dma_start(w_bounce[s * 128:(s + 1) * 128, :], vw[:])

            # ---- all-gather quantized weight shards (concat on dim 0)
            if nc_cores > 1:
                nc.gpsimd.collective_compute(
                    "AllGather",
                    mybir.AluOpType.bypass,
                    replica_groups=[list(range(nc_cores))],
                    ins=[w_bounce.opt()],
                    outs=[w_full.opt()],
                )
                wq_src = w_full
            else:
                wq_src = w_bounce

            # ---- quantize local input shard, transposed layout [k, m_sh]
            vx = []
            for s in range(kt):
                xr = xraw_pool.tile([128, m_sh], mybir.dt.bfloat16, tag="xraw",
                                    name=f"xr_{s}")
                nc.sync.dma_start(xr[:], xT[s * 128:(s + 1) * 128, :])
                v = vx_pool.tile([128, m_sh], mybir.dt.bfloat16, tag="vx",
                                 name=f"vx_{s}")
                _emit_quant(nc, pools, xr[:], v[:])
                vx.append(v)

            # ---- matmul: out[m, n] = 0.25 * (Vx.T @ Vw) + bias
            # wq_src viewed as [nc_cores(n-shards), k, n_sh]; rhs tile for
            # (global n-strip ns, k-tile s) is rows [base + 128*s, ...).
            for ns in range(nst):
                # which (core-shard, column range) this n-strip maps to
                strip_rhs = []
                for s in range(kt):
                    rt = rhs_pool.tile([128, n_strip], mybir.dt.bfloat16,
                                       tag="rhs", name=f"rhs_{ns}_{s}")
                    g0 = ns * n_strip          # global n offset
                    shard = g0 // n_sh if nc_cores > 1 else 0
                    col = g0 - shard * n_sh
                    base = shard * k if nc_cores > 1 else 0
                    nc.sync.dma_start(
                        rt[:],
                        wq_src[base + s * 128: base + (s + 1) * 128,
                               col: col + n_strip],
                    )
                    strip_rhs.append(rt)
                for mi in range(mt):
                    ps = psum_pool.tile([128, n_strip], mybir.dt.float32,
                                        tag="ps", name=f"ps_{ns}_{mi}")
                    for s in range(kt):
                        nc.tensor.matmul(
                            ps[:],
                            vx[s][:, mi * 128:(mi + 1) * 128],
                            strip_rhs[s][:],
                            start=(s == 0),
                            stop=False,
                        )
                    # += broadcast of 4*bias via K=1 matmul (exact in fp32)
                    nc.tensor.matmul(
                        ps[:],
                        ones_t[0:1, :],
                        bias_t[0:1, ns * n_strip:(ns + 1) * n_strip],
                        start=False,
                        stop=True,
                    )
                    ot = osb_pool.tile([128, n_strip], mybir.dt.bfloat16,
                                       tag="osb", name=f"ot_{ns}_{mi}")
                    nc.scalar.activation(ot[:], ps[:],
                                         mybir.ActivationFunctionType.Copy,
                                         scale=0.25)
                    nc.sync.dma_start(
                        out[mi * 128:(mi + 1) * 128,
                            ns * n_strip:(ns + 1) * n_strip],
                        ot[:],
                    )

    nc.compile()
    return nc


_PROGRAM_CACHE = {}


def _get_program(key, **kw):
    if key not in _PROGRAM_CACHE:
        _PROGRAM_CACHE[key] = build_program(**kw)
    return _PROGRAM_CACHE[key]


# ---------------------------------------------------------------- entry


def _numpy_fallback(input, weight, bias, scale_input, scale_weight):
    """Generic-scale reference path (host); only used if scales != 1."""
    BLOCK = 32
    GRID = np.array([0, .5, 1, 1.5, 2, 3, 4, 6], np.float32)
    MIDS = np.array([.25, .75, 1.25, 1.75, 2.5, 3.5, 5], np.float32)

    def dq(x, scale):
        shape = x.shape
        flat = x.reshape(-1).astype(np.float32)
        nels = flat.shape[0]
        pad = (-nels) % BLOCK
        flat = np.pad(flat, (0, pad))
        blk = flat.reshape(-1, BLOCK) / scale[:, None]
        idx = np.searchsorted(MIDS, np.abs(blk))
        q = np.sign(blk) * GRID[idx]
        return (q * scale[:, None]).reshape(-1)[:nels].reshape(shape)

    xq = dq(input, scale_input)
    wq = dq(weight, scale_weight)
    y = xq @ wq.T + bias.astype(np.float32)
    return y.astype(BF16)


def _install_profile_hook():
    """Provide antenv.axon_hooks (absent on this image) so trace=True works."""
    import sys, types, ctypes, contextlib

    if "antenv.axon_hooks" in sys.modules:
        return
    try:
        lib = ctypes.CDLL("/opt/axon/libaxon_pjrt.so")
        if not hasattr(lib, "axon_start_nrt_profile"):
            return
    except OSError:
        return
    lib.axon_start_nrt_profile.argtypes = [ctypes.POINTER(ctypes.c_int64), ctypes.c_size_t]
    lib.axon_start_nrt_profile.restype = ctypes.c_int64
    lib.axon_stop_nrt_profile.argtypes = [ctypes.c_char_p]
    lib.axon_stop_nrt_profile.restype = ctypes.c_int64

    @contextlib.contextmanager
    def _hook(output_dir, device_ids):
        import jax
        jax.devices()
        if device_ids:
            ids = (ctypes.c_int64 * len(device_ids))(*device_ids)
            rc = lib.axon_start_nrt_profile(ids, len(device_ids))
        else:
            rc = lib.axon_start_nrt_profile(None, 0)
        if rc != 0:
            raise RuntimeError(f"axon_start_nrt_profile rc={rc}")
        try:
            yield
        finally:
            n = lib.axon_stop_nrt_profile(str(output_dir).encode())
            print(f"profile: {n} file(s) written to {output_dir}", file=sys.stderr)

    mod = types.ModuleType("antenv.axon_hooks")
    mod.get_axon_ntff_profile_hook = lambda: _hook
    mod.set_axon_ntff_profile_hook = lambda h: None
    sys.modules["antenv.axon_hooks"] = mod
    try:
        import antenv
        antenv.axon_hooks = mod
    except ImportError:
        pass


def run(inputs, trace=False, **spmd_kwargs):
    """Returns (y, BassKernelResults)."""
    if trace:
        _install_profile_hook()
    x = np.asarray(inputs["input"])
    w = np.asarray(inputs["weight"])
    b = np.asarray(inputs["bias"])

    m_sh, n_sh = M // NC, N // NC
    xT = np.ascontiguousarray(np.asarray(x, BF16).T)           # [K, M]
    wT = np.ascontiguousarray(np.asarray(w, BF16).T)           # [K, N]
    bias4 = (np.asarray(b).astype(np.float32) * 4.0).astype(BF16)[None, :]

    nc = _get_program("full")
    in_maps = [
        {
            "xT": np.ascontiguousarray(xT[:, c * m_sh:(c + 1) * m_sh]),
            "wT": np.ascontiguousarray(wT[:, c * n_sh:(c + 1) * n_sh]),
            "bias4": bias4,
        }
        for c in range(NC)
    ]
    res = bass_utils.run_bass_kernel_spmd(
        nc, in_maps, core_ids=list(range(NC)), trace=trace, **spmd_kwargs
    )
    outs = [res.results[c]["out"] for c in range(NC)]
    y = np.concatenate(outs, axis=0).astype(BF16)
    return y, res


def kernel(**inputs):
    si = np.asarray(inputs["scale_input"])
    sw = np.asarray(inputs["scale_weight"])
    if not (np.all(si == 1.0) and np.all(sw == 1.0)):
        return _numpy_fallback(
            np.asarray(inputs["input"]), np.asarray(inputs["weight"]),
            np.asarray(inputs["bias"]), si, sw)
    y, _ = run(inputs)
    return y


if __name__ == "__main__":
    rng = np.random.default_rng(0)
    xs = rng.standard_normal((M, K)).astype(BF16)
    ws = (rng.standard_normal((N, K)) / np.sqrt(K)).astype(BF16)
    bs = rng.standard_normal(N).astype(BF16)
    ones_i = np.ones(M * K // 32, np.float32)
    ones_w = np.ones(N * K // 32, np.float32)
    y = kernel(input=xs, weight=ws, bias=bs, scale_input=ones_i, scale_weight=ones_w)
    print(y.shape, y.dtype)
